# revision 74
# baseline (speedup 1.0000x reference)
"""DenseGCNConv on 8 Trainium2 NeuronCores (Bass/Tile), mixed fp8 adj.

out = (adj @ features) @ W.T + b,  adj [16384,16384] f32 uniform[0,1),
features [16384,128], W [128,128], b [128].

Row-parallel: core c owns rows [c*2048, (c+1)*2048) of adj; out = adj @ fw
+ b with fw = features @ W.T precomputed on the host. Memory-bound on
streaming adj (32 MiB/core at 1 B/elem), so adj is quantized host-side:
adj = 0.5 + q/16 (centering halves quantization error; the 0.5*colsum
correction folds into the bias).

Mixed-precision k-split to balance TensorE against the DMA roofline:
  - DR region (first NDR*256 k-rows): e4m3 adj x e4m3 fw via DoubleRow
    matmuls (k=256/instruction, 2x flops -> 157 TF/s). Slightly noisier
    (3-bit mantissa) but twice the PE throughput.
  - e3 region (rest): e3m4 adj x f16 fw classic matmuls (78 TF/s).
Both regions accumulate at scale 256 (= 16 adj x 16 fw) into the same
PSUM banks; one ACT pass applies 1/256 + bias and writes f16 output
(upcast on host). NDR=14 -> rel err 1.365e-2, PE busy ~100us, DMA ~105us.

Schedule (measured-driven, see session notes): ALL adj groups ride the
sync HWDGE ring - one FIFO means delivery lands exactly in schedule
order, immune to the cross-ring descriptor-push race that caused
bimodal 130/143us runs. fw pieces, constants, and the output ride the
scalar ring so they are never head-of-line blocked behind a
buffer-full adj dma (each fw piece is issued FW_EARLY groups before
its first-use group, which also keeps the dep in program order - the
Tile framework cannot wait on a dma issued later). A tiny warm tile
lands first and feeds WARMUP dummy matmuls so the PE p-state ramps to
full clock (216ns/512-row) during NEFF boot (~9us of fixed
boot+dispatch+DGE latency before any bytes move). Ramp groups
[2,2,4 chunks] ease the PE in; steady 2 MiB groups (16 KiB descriptor
runs) follow; the 4 DoubleRow groups sit late in the stream where the
SBUF backlog absorbs their 2x consumption rate; small tail groups give
a soft landing after the DMA stream drains. Every early group gets its
own pool slot - a ramp pool with fewer bufs than uses parks the ring
and staircases the whole early phase. The last group finishes one
m-block at a time so bias-add + output DMA overlap remaining matmuls.
"""

import sys

if "/opt/trn_rl_repo" not in sys.path:
    sys.path.insert(0, "/opt/trn_rl_repo")

import ml_dtypes
import numpy as np

N = 16384
F = 128
P = 128
CORES = 8
ROWS = N // CORES  # 2048 adj rows per core
S = 16.0  # quant scale for both adj (centered) and fw

NDR = 14  # DoubleRow chunks of 256 k-rows (k in [0, NDR*256))
KDR = NDR * 256
E3C = (N - KDR) // P  # e3m4 chunks of 128 k-rows
RAMP = [2, 2, 4]  # head e3 group sizes (chunks)
TAIL = [2, 2]  # closing e3 group sizes (soft landing after DMA ends)
CK = 8  # steady e3 group size (2 MiB)
DR_SIZES = [4, 4, 4, 2]  # DR group sizes (256-row chunks)
DR_AFTER = [11, 12, 13, 14]  # insert DR group g before this-indexed e3 group
WARMUP = 8  # dummy matmuls during NEFF boot to ramp the PE p-state
FW_PIECE = 13  # e3 chunks per fw16 piece
FW_EARLY = 3  # issue fw piece i this many groups before its first-use group
# All adj rides the sync ring (one FIFO -> delivery exactly in schedule
# order, no cross-ring push race); fw/const/output ride the scalar ring.
ADJ8_BUFS = 6
ADJDR_BUFS = 2

_cache = {}


def configure(**kw):
    """Experiment knob: override module globals, invalidate build cache."""
    g = globals()
    for k, v in kw.items():
        assert k in g, k
        g[k] = v
    g["KDR"] = g["NDR"] * 256
    g["E3C"] = (N - g["KDR"]) // P
    _cache.clear()


def _split_excess_waits(nc, max_waits=1):
    """Walrus CoreV3 codegen rejects instructions with more than one SyncWait
    ("Too many sync wait commands"). Tile's kernel-tail drain accumulates one
    wait per semaphore lane; hoist the excess onto same-engine NoOps placed
    immediately before the offending instruction."""
    import concourse.mybir as mybir

    counter = [0]

    def fresh_name():
        counter[0] += 1
        return f"I-waitsplit-{counter[0]}"

    for fn in nc.m.functions:
        for blk in fn.blocks:
            new_insts = []
            for inst in blk.instructions:
                si = inst.sync_info
                if si is not None and si.on_wait and len(si.on_wait) > max_waits:
                    waits = list(si.on_wait)
                    extra, keep = waits[:-max_waits], waits[-max_waits:]
                    for i in range(0, len(extra), max_waits):
                        nop = mybir.InstNoOp(
                            name=fresh_name(),
                            engine=inst.engine,
                            sync_info=mybir.SyncInfo(
                                on_wait=extra[i : i + max_waits], on_update=[]
                            ),
                            bass_nofuse=True,
                        )
                        new_insts.append(nop)
                    si.on_wait = keep
                new_insts.append(inst)
            blk.instructions[:] = new_insts


def _schedule():
    """Group list: (kind, base_chunk, n_chunks). kind 'e3' or 'dr'."""
    sizes = list(RAMP)
    rem = E3C - sum(sizes) - sum(TAIL)
    assert rem % CK == 0, (E3C, sizes, CK)
    sizes += [CK] * (rem // CK) + list(TAIL)
    assert sum(DR_SIZES) == NDR
    sched = []
    c = 0
    e3_groups = 0
    dr_next = 0
    dr_base = 0
    for sz in sizes:
        while dr_next < len(DR_SIZES) and e3_groups == DR_AFTER[dr_next]:
            sched.append(("dr", dr_base, DR_SIZES[dr_next]))
            dr_base += DR_SIZES[dr_next]
            dr_next += 1
        sched.append(("e3", c, sz))
        c += sz
        e3_groups += 1
    assert c == E3C and dr_next == len(DR_SIZES), (c, dr_next)
    assert sched[-1][0] == "e3"
    return sched


def _build(split_waits=True):
    import concourse.bass as bass
    import concourse.mybir as mybir
    from concourse.tile import TileContext

    f32 = mybir.dt.float32
    f16 = mybir.dt.float16
    e4 = mybir.dt.float8e4
    e3 = mybir.dt.float8e3
    DRM = mybir.MatmulPerfMode.DoubleRow
    nc = bass.Bass()

    # adj3[p, c*ROWS + m] = q3[(KDR + c*128 + p), m] (e3m4, chunk-major cols)
    adj3 = nc.declare_dram_parameter("adj3", [P, E3C * ROWS], e3, isOutput=False)
    # adj4: DR chunk c, plane i: cols [c*2*ROWS + i*ROWS + m] = q4[c*256+i*128+p, m]
    adj4 = nc.declare_dram_parameter("adj4", [P, NDR * 2 * ROWS], e4, isOutput=False)
    # fw16[p, c*F + fo] = f16(S * fw[KDR + c*128 + p, fo])
    fw16 = nc.declare_dram_parameter("fw16", [P, E3C * F], f16, isOutput=False)
    # fwab[p, c*2F + i*F + fo] = e4m3(S * fw[c*256 + i*128 + p, fo])
    fwab = nc.declare_dram_parameter("fwab", [P, NDR * 2 * F], e4, isOutput=False)
    warm = nc.declare_dram_parameter("warm", [P, 512], e3, isOutput=False)
    # scaled-only output (bias applied host-side: one less tail dependency)
    outT = nc.declare_dram_parameter("outT", [P, ROWS], f16, isOutput=True)

    sched = _schedule()
    total_ck = E3C + NDR  # accumulation steps: e3 chunks + DR chunks
    n_fw_pieces = (E3C + FW_PIECE - 1) // FW_PIECE

    with TileContext(nc) as tc:
        with (
            tc.tile_pool(name="const", bufs=1) as const_pool,
            tc.tile_pool(name="fw", bufs=1) as fw_pool,
            tc.tile_pool(name="ramp2", bufs=2) as ramp2_pool,
            tc.tile_pool(name="ramp4", bufs=RAMP.count(4)) as ramp4_pool,
            tc.tile_pool(name="adj8", bufs=ADJ8_BUFS) as adj8_pool,
            tc.tile_pool(name="adjdr", bufs=ADJDR_BUFS) as adjdr_pool,
            tc.tile_pool(name="outp", bufs=1) as out_pool,
            tc.tile_pool(name="ps", bufs=1, space="PSUM") as ps_pool,
        ):
            pools = {2: ramp2_pool, 4: ramp4_pool, CK: adj8_pool}

            fw_tiles = [
                fw_pool.tile(
                    [P, min(FW_PIECE, E3C - i * FW_PIECE) * F],
                    f16,
                    name=f"fwt{i}",
                    tag=f"fw{i}",
                )
                for i in range(n_fw_pieces)
            ]
            fwab_t = fw_pool.tile([P, NDR * 2 * F], e4)
            o_sb = out_pool.tile([P, ROWS], f16)
            po = [
                ps_pool.tile([P, 512], f32, name=f"po{mb}", tag=f"po{mb}")
                for mb in range(4)
            ]

            fw_next = [0]

            def fw_dma(i):
                c0 = i * FW_PIECE
                ncols = fw_tiles[i].shape[1]
                nc.scalar.dma_start(
                    out=fw_tiles[i], in_=fw16[:, c0 * F : c0 * F + ncols]
                )

            # schedule position (group index) of each e3 chunk's group
            chunk_group = {}
            for pos, (kind, base, gck) in enumerate(sched):
                if kind == "e3":
                    for j in range(gck):
                        chunk_group[base + j] = pos
            # fw piece i -> schedule position after which to issue its dma
            fw_issue_at = {}
            for i in range(n_fw_pieces):
                pos = max(0, chunk_group[i * FW_PIECE] - FW_EARLY)
                fw_issue_at.setdefault(pos, []).append(i)

            # --- issue order: first bytes needed first ---
            # ramp group 0 adj, then fw piece 0, then the rest
            gidx = 0
            step = [0]  # global accumulation step

            def mm_e3(at, j_local, c_global, mb):
                piece, off = divmod(c_global, FW_PIECE)
                nc.tensor.matmul(
                    po[mb],
                    lhsT=fw_tiles[piece][:, off * F : (off + 1) * F],
                    rhs=at[:, j_local * ROWS + mb * 512 : j_local * ROWS + (mb + 1) * 512],
                    start=(step[0] == 0),
                    stop=(step[0] == total_ck - 1),
                )

            def mm_dr(at, j_local, c_global, mb):
                w = fwab_t[:, c_global * 2 * F : (c_global + 1) * 2 * F].rearrange(
                    "p (two f) -> p two f", two=2
                )
                rhs = at[:, j_local * 2 * ROWS : (j_local + 1) * 2 * ROWS].rearrange(
                    "p (two m) -> p two m", two=2
                )[:, :, mb * 512 : (mb + 1) * 512]
                nc.tensor.matmul(
                    po[mb],
                    lhsT=w,
                    rhs=rhs,
                    start=(step[0] == 0),
                    stop=(step[0] == total_ck - 1),
                    perf_mode=DRM,
                )

            # --- preamble ---
            # p-state warmup reads a tiny tile landing first on the fw ring;
            # dummy matmuls run at low/mid clock during boot so real matmuls
            # start at full speed. po[0] is reset by its later start=True.
            warm_t = const_pool.tile([P, 512], e3)
            nc.scalar.dma_start(out=warm_t, in_=warm[:])
            for _ in range(WARMUP):
                nc.tensor.matmul(
                    po[0], lhsT=warm_t[:, :128], rhs=warm_t[:], start=True, stop=True
                )
            fw_dma(0)
            fw_next[0] = 1

            def adj3_dma(base, gck):
                at = pools[gck].tile([P, gck * ROWS], e3, name=f"a{gck}", tag=f"a{gck}")
                nc.sync.dma_start(
                    out=at, in_=adj3[:, base * ROWS : (base + gck) * ROWS]
                )
                return at

            for gi, (kind, base, gck) in enumerate(sched):
                last = gi == len(sched) - 1
                if kind == "e3":
                    at = adj3_dma(base, gck)
                    if gi == 8:
                        # fwab is small (0.44 MiB) and needed from the first
                        # DR group; issue a few groups early on the fw ring
                        nc.scalar.dma_start(out=fwab_t, in_=fwab[:])
                    for i in fw_issue_at.get(gi, []):
                        if i >= fw_next[0]:
                            fw_dma(i)
                            fw_next[0] = i + 1
                    if not last:
                        for j in range(gck):
                            for mb in range(4):
                                mm_e3(at, j, base + j, mb)
                            step[0] += 1
                    else:
                        sv = step[0]
                        for mb in range(4):
                            for j in range(gck):
                                step[0] = sv + j
                                mm_e3(at, j, base + j, mb)
                            sl = slice(mb * 512, (mb + 1) * 512)
                            nc.scalar.mul(o_sb[:, sl], po[mb], 1.0 / (S * S))
                            nc.scalar.dma_start(out=outT[:, sl], in_=o_sb[:, sl])
                        step[0] = sv + gck
                else:
                    at = adjdr_pool.tile(
                        [P, gck * 2 * ROWS], e4, name=f"adr{gck}", tag=f"adr{gck}"
                    )
                    nc.sync.dma_start(
                        out=at, in_=adj4[:, base * 2 * ROWS : (base + gck) * 2 * ROWS]
                    )
                    for i in fw_issue_at.get(gi, []):
                        if i >= fw_next[0]:
                            fw_dma(i)
                            fw_next[0] = i + 1
                    for j in range(gck):
                        for mb in range(4):
                            mm_dr(at, j, base + j, mb)
                        step[0] += 1
            assert step[0] == total_ck

    if split_waits:
        _split_excess_waits(nc)
    return nc


def _get_nc():
    if "nc" not in _cache:
        _cache["nc"] = _build()
    return _cache["nc"]


def make_in_maps(adj, features, W, b):
    adj = np.asarray(adj, dtype=np.float32)
    features = np.asarray(features, dtype=np.float32)
    W = np.asarray(W, dtype=np.float32)
    b = np.asarray(b, dtype=np.float32)

    fw = features @ W.T  # [N, F] f32
    sfw = fw * np.float32(S)
    fwab_rows = sfw[:KDR].astype(ml_dtypes.float8_e4m3)  # [KDR, F]
    fw16_rows = sfw[KDR:].astype(np.float16)  # [N-KDR, F]

    # fwab[p, c*2F + i*F + fo] = fwab_rows[c*256 + i*128 + p, fo]
    fwab = np.ascontiguousarray(
        fwab_rows.reshape(NDR, 2, P, F).transpose(2, 0, 1, 3)
    ).reshape(P, NDR * 2 * F)
    # fw16[p, c*F + fo] = fw16_rows[c*128 + p, fo]
    fw16 = np.ascontiguousarray(
        fw16_rows.reshape(E3C, P, F).transpose(1, 0, 2)
    ).reshape(P, E3C * F)

    # bias' = b + 0.5 * colsum(decoded effective fw), exact in f64; applied
    # host-side in assemble_output (the device ships raw scale-256 PSUM)
    fw_eff = np.concatenate(
        [
            fwab_rows.astype(np.float64) / S,
            fw16_rows.astype(np.float64) / S,
        ]
    )
    _cache["bias"] = (b.astype(np.float64) + 0.5 * fw_eff.sum(axis=0)).astype(
        np.float32
    )

    in_maps = []
    for c in range(CORES):
        sh = adj[c * ROWS : (c + 1) * ROWS, :]  # [ROWS, N]
        cen = (sh - np.float32(0.5)) * np.float32(S)
        q4 = cen[:, :KDR].astype(ml_dtypes.float8_e4m3)  # [ROWS, KDR]
        q3 = cen[:, KDR:].astype(ml_dtypes.float8_e3m4)  # [ROWS, N-KDR]
        # adj4[p, c*2*ROWS + i*ROWS + m] = q4.T[c*256 + i*128 + p, m]
        a4 = np.ascontiguousarray(
            q4.T.reshape(NDR, 2, P, ROWS).transpose(2, 0, 1, 3)
        ).reshape(P, NDR * 2 * ROWS)
        # adj3[p, c*ROWS + m] = q3.T[c*128 + p, m]
        a3 = np.ascontiguousarray(
            q3.T.reshape(E3C, P, ROWS).transpose(1, 0, 2)
        ).reshape(P, E3C * ROWS)
        in_maps.append(
            {
                "adj3": a3,
                "adj4": a4,
                "fw16": fw16,
                "fwab": fwab,
                "warm": np.zeros((P, 512), ml_dtypes.float8_e3m4),
            }
        )
    return in_maps


def assemble_output(results):
    bias = _cache["bias"]  # set by make_in_maps
    out = np.empty((N, F), dtype=np.float32)
    for c in range(CORES):
        out[c * ROWS : (c + 1) * ROWS, :] = (
            results[c]["outT"].astype(np.float32).T + bias
        )
    return out


def kernel(adj, features, W, b):
    from concourse.bass_utils import run_bass_kernel_spmd

    nc = _get_nc()
    in_maps = make_in_maps(adj, features, W, b)
    res = run_bass_kernel_spmd(nc, in_maps, list(range(CORES)))
    return assemble_output(res.results)


# revision 77
# speedup vs baseline: 1.1316x; 1.1316x over previous
"""DenseGCNConv on 8 Trainium2 NeuronCores (Bass/Tile), mixed fp8 adj.

out = (adj @ features) @ W.T + b,  adj [16384,16384] f32 uniform[0,1),
features [16384,128], W [128,128], b [128].

Row-parallel: core c owns rows [c*2048, (c+1)*2048) of adj; out = adj @ fw
+ b with fw = features @ W.T precomputed on the host. Memory-bound on
streaming adj (32 MiB/core at 1 B/elem), so adj is quantized host-side:
adj = 0.5 + q/16 (centering halves quantization error; the 0.5*colsum
correction folds into the bias).

Mixed-precision k-split to balance TensorE against the DMA roofline:
  - DR region (first NDR*256 k-rows): e4m3 adj x e4m3 fw via DoubleRow
    matmuls (k=256/instruction, 2x flops -> 157 TF/s). Slightly noisier
    (3-bit mantissa) but twice the PE throughput.
  - e3 region (rest): e3m4 adj x f16 fw classic matmuls (78 TF/s).
Both regions accumulate at scale 256 (= 16 adj x 16 fw) into the same
PSUM banks; one ACT pass applies 1/256 + bias and writes f16 output
(upcast on host). NDR=12 -> rel err ~1.2e-2, PE busy ~100us, DMA ~105us.

Schedule: ramp groups [1,1,2,4 chunks] start the PE ~4us in; steady
2 MiB groups (16 KiB descriptor runs) alternate the two HWDGE rings;
3 DoubleRow groups interleave mid-stream; fw pieces ride ahead of first
use; bias on the gpsimd queue. Last group finishes one m-block at a
time so bias-add + output DMA overlap the remaining matmuls.
"""

import sys

if "/opt/trn_rl_repo" not in sys.path:
    sys.path.insert(0, "/opt/trn_rl_repo")

import ml_dtypes
import numpy as np

N = 16384
F = 128
P = 128
CORES = 8
ROWS = N // CORES  # 2048 adj rows per core
S = 16.0  # quant scale for both adj (centered) and fw

NDR = 14  # DoubleRow chunks of 256 k-rows (k in [0, NDR*256))
KDR = NDR * 256
E3C = (N - KDR) // P  # e3m4 chunks of 128 k-rows
RAMP = [2, 2, 4]  # head e3 group sizes (chunks)
TAIL = [2, 2]  # closing e3 group sizes (soft landing after DMA ends)
CK = 8  # steady e3 group size (2 MiB)
DR_SIZES = [4, 4, 4, 2]  # DR group sizes (256-row chunks)
DR_AFTER = [11, 12, 13, 14]  # insert DR group g before this-indexed e3 group
WARMUP = 8  # dummy matmuls during NEFF boot to ramp the PE p-state
FW_PIECE = 13  # e3 chunks per fw16 piece
FW_EARLY = 3  # issue fw piece i this many groups before its first-use group
# All adj rides the sync ring (one FIFO -> delivery exactly in schedule
# order, no cross-ring push race); fw/const/output ride the scalar ring.
ADJ8_BUFS = 6
ADJDR_BUFS = 2

_cache = {}


def configure(**kw):
    """Experiment knob: override module globals, invalidate build cache."""
    g = globals()
    for k, v in kw.items():
        assert k in g, k
        g[k] = v
    g["KDR"] = g["NDR"] * 256
    g["E3C"] = (N - g["KDR"]) // P
    _cache.clear()


def _split_excess_waits(nc, max_waits=1):
    """Walrus CoreV3 codegen rejects instructions with more than one SyncWait
    ("Too many sync wait commands"). Tile's kernel-tail drain accumulates one
    wait per semaphore lane; hoist the excess onto same-engine NoOps placed
    immediately before the offending instruction."""
    import concourse.mybir as mybir

    counter = [0]

    def fresh_name():
        counter[0] += 1
        return f"I-waitsplit-{counter[0]}"

    for fn in nc.m.functions:
        for blk in fn.blocks:
            new_insts = []
            for inst in blk.instructions:
                si = inst.sync_info
                if si is not None and si.on_wait and len(si.on_wait) > max_waits:
                    waits = list(si.on_wait)
                    extra, keep = waits[:-max_waits], waits[-max_waits:]
                    for i in range(0, len(extra), max_waits):
                        nop = mybir.InstNoOp(
                            name=fresh_name(),
                            engine=inst.engine,
                            sync_info=mybir.SyncInfo(
                                on_wait=extra[i : i + max_waits], on_update=[]
                            ),
                            bass_nofuse=True,
                        )
                        new_insts.append(nop)
                    si.on_wait = keep
                new_insts.append(inst)
            blk.instructions[:] = new_insts


def _schedule():
    """Group list: (kind, base_chunk, n_chunks). kind 'e3' or 'dr'."""
    sizes = list(RAMP)
    rem = E3C - sum(sizes) - sum(TAIL)
    assert rem % CK == 0, (E3C, sizes, CK)
    sizes += [CK] * (rem // CK) + list(TAIL)
    assert sum(DR_SIZES) == NDR
    sched = []
    c = 0
    e3_groups = 0
    dr_next = 0
    dr_base = 0
    for sz in sizes:
        while dr_next < len(DR_SIZES) and e3_groups == DR_AFTER[dr_next]:
            sched.append(("dr", dr_base, DR_SIZES[dr_next]))
            dr_base += DR_SIZES[dr_next]
            dr_next += 1
        sched.append(("e3", c, sz))
        c += sz
        e3_groups += 1
    assert c == E3C and dr_next == len(DR_SIZES), (c, dr_next)
    assert sched[-1][0] == "e3"
    return sched


def _build(split_waits=True):
    import concourse.bass as bass
    import concourse.mybir as mybir
    from concourse.tile import TileContext

    f32 = mybir.dt.float32
    f16 = mybir.dt.float16
    e4 = mybir.dt.float8e4
    e3 = mybir.dt.float8e3
    DRM = mybir.MatmulPerfMode.DoubleRow
    nc = bass.Bass()

    # adj3[p, c*ROWS + m] = q3[(KDR + c*128 + p), m] (e3m4, chunk-major cols)
    adj3 = nc.declare_dram_parameter("adj3", [P, E3C * ROWS], e3, isOutput=False)
    # adj4: DR chunk c, plane i: cols [c*2*ROWS + i*ROWS + m] = q4[c*256+i*128+p, m]
    adj4 = nc.declare_dram_parameter("adj4", [P, NDR * 2 * ROWS], e4, isOutput=False)
    # fw16[p, c*F + fo] = f16(S * fw[KDR + c*128 + p, fo])
    fw16 = nc.declare_dram_parameter("fw16", [P, E3C * F], f16, isOutput=False)
    # fwab[p, c*2F + i*F + fo] = e4m3(S * fw[c*256 + i*128 + p, fo])
    fwab = nc.declare_dram_parameter("fwab", [P, NDR * 2 * F], e4, isOutput=False)
    bias = nc.declare_dram_parameter("bias", [P, 1], f32, isOutput=False)
    warm = nc.declare_dram_parameter("warm", [P, 512], e3, isOutput=False)
    outT = nc.declare_dram_parameter("outT", [P, ROWS], f16, isOutput=True)

    sched = _schedule()
    total_ck = E3C + NDR  # accumulation steps: e3 chunks + DR chunks
    n_fw_pieces = (E3C + FW_PIECE - 1) // FW_PIECE

    with TileContext(nc) as tc:
        with (
            tc.tile_pool(name="const", bufs=1) as const_pool,
            tc.tile_pool(name="fw", bufs=1) as fw_pool,
            tc.tile_pool(name="ramp2", bufs=2) as ramp2_pool,
            tc.tile_pool(name="ramp4", bufs=RAMP.count(4)) as ramp4_pool,
            tc.tile_pool(name="adj8", bufs=ADJ8_BUFS) as adj8_pool,
            tc.tile_pool(name="adjdr", bufs=ADJDR_BUFS) as adjdr_pool,
            tc.tile_pool(name="outp", bufs=1) as out_pool,
            tc.tile_pool(name="ps", bufs=1, space="PSUM") as ps_pool,
        ):
            pools = {2: ramp2_pool, 4: ramp4_pool, CK: adj8_pool}

            fw_tiles = [
                fw_pool.tile(
                    [P, min(FW_PIECE, E3C - i * FW_PIECE) * F],
                    f16,
                    name=f"fwt{i}",
                    tag=f"fw{i}",
                )
                for i in range(n_fw_pieces)
            ]
            fwab_t = fw_pool.tile([P, NDR * 2 * F], e4)
            b_sb = const_pool.tile([P, 1], f32)
            o_sb = out_pool.tile([P, ROWS], f16)
            po = [
                ps_pool.tile([P, 512], f32, name=f"po{mb}", tag=f"po{mb}")
                for mb in range(4)
            ]

            fw_next = [0]

            def fw_dma(i):
                c0 = i * FW_PIECE
                ncols = fw_tiles[i].shape[1]
                nc.scalar.dma_start(
                    out=fw_tiles[i], in_=fw16[:, c0 * F : c0 * F + ncols]
                )

            # schedule position (group index) of each e3 chunk's group
            chunk_group = {}
            for pos, (kind, base, gck) in enumerate(sched):
                if kind == "e3":
                    for j in range(gck):
                        chunk_group[base + j] = pos
            # fw piece i -> schedule position after which to issue its dma
            fw_issue_at = {}
            for i in range(n_fw_pieces):
                pos = max(0, chunk_group[i * FW_PIECE] - FW_EARLY)
                fw_issue_at.setdefault(pos, []).append(i)

            # --- issue order: first bytes needed first ---
            # ramp group 0 adj, then fw piece 0, then the rest
            gidx = 0
            step = [0]  # global accumulation step

            def mm_e3(at, j_local, c_global, mb):
                piece, off = divmod(c_global, FW_PIECE)
                nc.tensor.matmul(
                    po[mb],
                    lhsT=fw_tiles[piece][:, off * F : (off + 1) * F],
                    rhs=at[:, j_local * ROWS + mb * 512 : j_local * ROWS + (mb + 1) * 512],
                    start=(step[0] == 0),
                    stop=(step[0] == total_ck - 1),
                )

            def mm_dr(at, j_local, c_global, mb):
                w = fwab_t[:, c_global * 2 * F : (c_global + 1) * 2 * F].rearrange(
                    "p (two f) -> p two f", two=2
                )
                rhs = at[:, j_local * 2 * ROWS : (j_local + 1) * 2 * ROWS].rearrange(
                    "p (two m) -> p two m", two=2
                )[:, :, mb * 512 : (mb + 1) * 512]
                nc.tensor.matmul(
                    po[mb],
                    lhsT=w,
                    rhs=rhs,
                    start=(step[0] == 0),
                    stop=(step[0] == total_ck - 1),
                    perf_mode=DRM,
                )

            # --- preamble ---
            # p-state warmup reads a tiny tile landing first on the fw ring;
            # dummy matmuls run at low/mid clock during boot so real matmuls
            # start at full speed. po[0] is reset by its later start=True.
            warm_t = const_pool.tile([P, 512], e3)
            nc.scalar.dma_start(out=warm_t, in_=warm[:])
            for _ in range(WARMUP):
                nc.tensor.matmul(
                    po[0], lhsT=warm_t[:, :128], rhs=warm_t[:], start=True, stop=True
                )
            fw_dma(0)
            fw_next[0] = 1
            nc.scalar.dma_start(out=b_sb, in_=bias[:])

            def adj3_dma(base, gck):
                at = pools[gck].tile([P, gck * ROWS], e3, name=f"a{gck}", tag=f"a{gck}")
                nc.sync.dma_start(
                    out=at, in_=adj3[:, base * ROWS : (base + gck) * ROWS]
                )
                return at

            for gi, (kind, base, gck) in enumerate(sched):
                last = gi == len(sched) - 1
                if kind == "e3":
                    at = adj3_dma(base, gck)
                    if gi == 8:
                        # fwab is small (0.44 MiB) and needed from the first
                        # DR group; issue a few groups early on the fw ring
                        nc.scalar.dma_start(out=fwab_t, in_=fwab[:])
                    for i in fw_issue_at.get(gi, []):
                        if i >= fw_next[0]:
                            fw_dma(i)
                            fw_next[0] = i + 1
                    if not last:
                        for j in range(gck):
                            for mb in range(4):
                                mm_e3(at, j, base + j, mb)
                            step[0] += 1
                    else:
                        sv = step[0]
                        for mb in range(4):
                            for j in range(gck):
                                step[0] = sv + j
                                mm_e3(at, j, base + j, mb)
                            sl = slice(mb * 512, (mb + 1) * 512)
                            # out dma rides the (drained) sync ring so the
                            # scalar sequencer runs the four ACTs back-to-back
                            # instead of serializing act->dispatch->act->...
                            nc.scalar.activation(
                                o_sb[:, sl],
                                po[mb],
                                mybir.ActivationFunctionType.Identity,
                                bias=b_sb,
                                scale=1.0 / (S * S),
                            )
                            nc.sync.dma_start(out=outT[:, sl], in_=o_sb[:, sl])
                        step[0] = sv + gck
                else:
                    at = adjdr_pool.tile(
                        [P, gck * 2 * ROWS], e4, name=f"adr{gck}", tag=f"adr{gck}"
                    )
                    nc.sync.dma_start(
                        out=at, in_=adj4[:, base * 2 * ROWS : (base + gck) * 2 * ROWS]
                    )
                    for i in fw_issue_at.get(gi, []):
                        if i >= fw_next[0]:
                            fw_dma(i)
                            fw_next[0] = i + 1
                    for j in range(gck):
                        for mb in range(4):
                            mm_dr(at, j, base + j, mb)
                        step[0] += 1
            assert step[0] == total_ck

    if split_waits:
        _split_excess_waits(nc)
    return nc


def _get_nc():
    if "nc" not in _cache:
        _cache["nc"] = _build()
    return _cache["nc"]


def make_in_maps(adj, features, W, b):
    adj = np.asarray(adj, dtype=np.float32)
    features = np.asarray(features, dtype=np.float32)
    W = np.asarray(W, dtype=np.float32)
    b = np.asarray(b, dtype=np.float32)

    fw = features @ W.T  # [N, F] f32
    sfw = fw * np.float32(S)
    fwab_rows = sfw[:KDR].astype(ml_dtypes.float8_e4m3)  # [KDR, F]
    fw16_rows = sfw[KDR:].astype(np.float16)  # [N-KDR, F]

    # fwab[p, c*2F + i*F + fo] = fwab_rows[c*256 + i*128 + p, fo]
    fwab = np.ascontiguousarray(
        fwab_rows.reshape(NDR, 2, P, F).transpose(2, 0, 1, 3)
    ).reshape(P, NDR * 2 * F)
    # fw16[p, c*F + fo] = fw16_rows[c*128 + p, fo]
    fw16 = np.ascontiguousarray(
        fw16_rows.reshape(E3C, P, F).transpose(1, 0, 2)
    ).reshape(P, E3C * F)

    # bias' = b + 0.5 * colsum(decoded effective fw), exact in f64
    fw_eff = np.concatenate(
        [
            fwab_rows.astype(np.float64) / S,
            fw16_rows.astype(np.float64) / S,
        ]
    )
    bias = (b.astype(np.float64) + 0.5 * fw_eff.sum(axis=0)).astype(np.float32)
    bias = np.ascontiguousarray(bias.reshape(P, 1))

    in_maps = []
    for c in range(CORES):
        sh = adj[c * ROWS : (c + 1) * ROWS, :]  # [ROWS, N]
        cen = (sh - np.float32(0.5)) * np.float32(S)
        q4 = cen[:, :KDR].astype(ml_dtypes.float8_e4m3)  # [ROWS, KDR]
        q3 = cen[:, KDR:].astype(ml_dtypes.float8_e3m4)  # [ROWS, N-KDR]
        # adj4[p, c*2*ROWS + i*ROWS + m] = q4.T[c*256 + i*128 + p, m]
        a4 = np.ascontiguousarray(
            q4.T.reshape(NDR, 2, P, ROWS).transpose(2, 0, 1, 3)
        ).reshape(P, NDR * 2 * ROWS)
        # adj3[p, c*ROWS + m] = q3.T[c*128 + p, m]
        a3 = np.ascontiguousarray(
            q3.T.reshape(E3C, P, ROWS).transpose(1, 0, 2)
        ).reshape(P, E3C * ROWS)
        in_maps.append(
            {
                "adj3": a3,
                "adj4": a4,
                "fw16": fw16,
                "fwab": fwab,
                "bias": bias,
                "warm": np.zeros((P, 512), ml_dtypes.float8_e3m4),
            }
        )
    return in_maps


def assemble_output(results):
    out = np.empty((N, F), dtype=np.float32)
    for c in range(CORES):
        out[c * ROWS : (c + 1) * ROWS, :] = results[c]["outT"].astype(np.float32).T
    return out


def kernel(adj, features, W, b):
    from concourse.bass_utils import run_bass_kernel_spmd

    nc = _get_nc()
    in_maps = make_in_maps(adj, features, W, b)
    res = run_bass_kernel_spmd(nc, in_maps, list(range(CORES)))
    return assemble_output(res.results)


# revision 80
# speedup vs baseline: 1.1592x; 1.0244x over previous
"""DenseGCNConv on 8 Trainium2 NeuronCores (Bass/Tile), mixed fp8 adj.

out = (adj @ features) @ W.T + b,  adj [16384,16384] f32 uniform[0,1),
features [16384,128], W [128,128], b [128].

Row-parallel: core c owns rows [c*2048, (c+1)*2048) of adj; out = adj @ fw
+ b with fw = features @ W.T precomputed on the host. Memory-bound on
streaming adj (32 MiB/core at 1 B/elem), so adj is quantized host-side:
adj = 0.5 + q/16 (centering halves quantization error; the 0.5*colsum
correction folds into the bias).

Mixed-precision k-split to balance TensorE against the DMA roofline:
  - DR region (first NDR*256 k-rows): e4m3 adj x e4m3 fw via DoubleRow
    matmuls (k=256/instruction, 2x flops -> 157 TF/s). Slightly noisier
    (3-bit mantissa) but twice the PE throughput.
  - e3 region (rest): e3m4 adj x f16 fw classic matmuls (78 TF/s).
Both regions accumulate at scale 256 (= 16 adj x 16 fw) into the same
PSUM banks; one ACT pass applies 1/256 + bias and writes f16 output
(upcast on host). NDR=12 -> rel err ~1.2e-2, PE busy ~100us, DMA ~105us.

Schedule: ramp groups [1,1,2,4 chunks] start the PE ~4us in; steady
2 MiB groups (16 KiB descriptor runs) alternate the two HWDGE rings;
3 DoubleRow groups interleave mid-stream; fw pieces ride ahead of first
use; bias on the gpsimd queue. Last group finishes one m-block at a
time so bias-add + output DMA overlap the remaining matmuls.
"""

import sys

if "/opt/trn_rl_repo" not in sys.path:
    sys.path.insert(0, "/opt/trn_rl_repo")

import ml_dtypes
import numpy as np

N = 16384
F = 128
P = 128
CORES = 8
ROWS = N // CORES  # 2048 adj rows per core
S = 16.0  # quant scale for both adj (centered) and fw

NDR = 14  # DoubleRow chunks of 256 k-rows (k in [0, NDR*256))
KDR = NDR * 256
E3C = (N - KDR) // P  # e3m4 chunks of 128 k-rows
RAMP = [2, 2, 4, 4, 4]  # head e3 group sizes (chunks)
TAIL = [2, 2]  # closing e3 group sizes (soft landing after DMA ends)
CK = 8  # steady e3 group size (2 MiB)
DR_SIZES = [4, 4, 4, 2]  # DR group sizes (256-row chunks)
DR_AFTER = [12, 13, 14, 15]  # insert DR group g before this-indexed e3 group
WARMUP = 8  # dummy matmuls during NEFF boot to ramp the PE p-state
FW_PIECE = 13  # e3 chunks per fw16 piece
FW_EARLY = 3  # issue fw piece i this many groups before its first-use group
# All adj rides the sync ring (one FIFO -> delivery exactly in schedule
# order, no cross-ring push race); fw/const/output ride the scalar ring.
ADJ8_BUFS = 5
ADJDR_BUFS = 2

_cache = {}


def configure(**kw):
    """Experiment knob: override module globals, invalidate build cache."""
    g = globals()
    for k, v in kw.items():
        assert k in g, k
        g[k] = v
    g["KDR"] = g["NDR"] * 256
    g["E3C"] = (N - g["KDR"]) // P
    _cache.clear()


def _split_excess_waits(nc, max_waits=1):
    """Walrus CoreV3 codegen rejects instructions with more than one SyncWait
    ("Too many sync wait commands"). Tile's kernel-tail drain accumulates one
    wait per semaphore lane; hoist the excess onto same-engine NoOps placed
    immediately before the offending instruction."""
    import concourse.mybir as mybir

    counter = [0]

    def fresh_name():
        counter[0] += 1
        return f"I-waitsplit-{counter[0]}"

    for fn in nc.m.functions:
        for blk in fn.blocks:
            new_insts = []
            for inst in blk.instructions:
                si = inst.sync_info
                if si is not None and si.on_wait and len(si.on_wait) > max_waits:
                    waits = list(si.on_wait)
                    extra, keep = waits[:-max_waits], waits[-max_waits:]
                    for i in range(0, len(extra), max_waits):
                        nop = mybir.InstNoOp(
                            name=fresh_name(),
                            engine=inst.engine,
                            sync_info=mybir.SyncInfo(
                                on_wait=extra[i : i + max_waits], on_update=[]
                            ),
                            bass_nofuse=True,
                        )
                        new_insts.append(nop)
                    si.on_wait = keep
                new_insts.append(inst)
            blk.instructions[:] = new_insts


def _schedule():
    """Group list: (kind, base_chunk, n_chunks). kind 'e3' or 'dr'."""
    sizes = list(RAMP)
    rem = E3C - sum(sizes) - sum(TAIL)
    assert rem % CK == 0, (E3C, sizes, CK)
    sizes += [CK] * (rem // CK) + list(TAIL)
    assert sum(DR_SIZES) == NDR
    sched = []
    c = 0
    e3_groups = 0
    dr_next = 0
    dr_base = 0
    for sz in sizes:
        while dr_next < len(DR_SIZES) and e3_groups == DR_AFTER[dr_next]:
            sched.append(("dr", dr_base, DR_SIZES[dr_next]))
            dr_base += DR_SIZES[dr_next]
            dr_next += 1
        sched.append(("e3", c, sz))
        c += sz
        e3_groups += 1
    assert c == E3C and dr_next == len(DR_SIZES), (c, dr_next)
    assert sched[-1][0] == "e3"
    return sched


def _build(split_waits=True):
    import concourse.bass as bass
    import concourse.mybir as mybir
    from concourse.tile import TileContext

    f32 = mybir.dt.float32
    f16 = mybir.dt.float16
    e4 = mybir.dt.float8e4
    e3 = mybir.dt.float8e3
    DRM = mybir.MatmulPerfMode.DoubleRow
    nc = bass.Bass()

    # adj3[p, c*ROWS + m] = q3[(KDR + c*128 + p), m] (e3m4, chunk-major cols)
    adj3 = nc.declare_dram_parameter("adj3", [P, E3C * ROWS], e3, isOutput=False)
    # adj4: DR chunk c, plane i: cols [c*2*ROWS + i*ROWS + m] = q4[c*256+i*128+p, m]
    adj4 = nc.declare_dram_parameter("adj4", [P, NDR * 2 * ROWS], e4, isOutput=False)
    # fw16[p, c*F + fo] = f16(S * fw[KDR + c*128 + p, fo])
    fw16 = nc.declare_dram_parameter("fw16", [P, E3C * F], f16, isOutput=False)
    # fwab[p, c*2F + i*F + fo] = e4m3(S * fw[c*256 + i*128 + p, fo])
    fwab = nc.declare_dram_parameter("fwab", [P, NDR * 2 * F], e4, isOutput=False)
    bias = nc.declare_dram_parameter("bias", [P, 1], f32, isOutput=False)
    warm = nc.declare_dram_parameter("warm", [P, 512], e3, isOutput=False)
    outT = nc.declare_dram_parameter("outT", [P, ROWS], f16, isOutput=True)

    sched = _schedule()
    total_ck = E3C + NDR  # accumulation steps: e3 chunks + DR chunks
    n_fw_pieces = (E3C + FW_PIECE - 1) // FW_PIECE

    with TileContext(nc) as tc:
        with (
            tc.tile_pool(name="const", bufs=1) as const_pool,
            tc.tile_pool(name="fw", bufs=1) as fw_pool,
            tc.tile_pool(name="ramp2", bufs=2) as ramp2_pool,
            tc.tile_pool(name="ramp4", bufs=RAMP.count(4)) as ramp4_pool,
            tc.tile_pool(name="adj8", bufs=ADJ8_BUFS) as adj8_pool,
            tc.tile_pool(name="adjdr", bufs=ADJDR_BUFS) as adjdr_pool,
            tc.tile_pool(name="outp", bufs=1) as out_pool,
            tc.tile_pool(name="ps", bufs=1, space="PSUM") as ps_pool,
        ):
            pools = {2: ramp2_pool, 4: ramp4_pool, CK: adj8_pool}

            fw_tiles = [
                fw_pool.tile(
                    [P, min(FW_PIECE, E3C - i * FW_PIECE) * F],
                    f16,
                    name=f"fwt{i}",
                    tag=f"fw{i}",
                )
                for i in range(n_fw_pieces)
            ]
            fwab_t = fw_pool.tile([P, NDR * 2 * F], e4)
            b_sb = const_pool.tile([P, 1], f32)
            o_sb = out_pool.tile([P, ROWS], f16)
            po = [
                ps_pool.tile([P, 512], f32, name=f"po{mb}", tag=f"po{mb}")
                for mb in range(4)
            ]

            fw_next = [0]

            def fw_dma(i):
                c0 = i * FW_PIECE
                ncols = fw_tiles[i].shape[1]
                nc.scalar.dma_start(
                    out=fw_tiles[i], in_=fw16[:, c0 * F : c0 * F + ncols]
                )

            # schedule position (group index) of each e3 chunk's group
            chunk_group = {}
            for pos, (kind, base, gck) in enumerate(sched):
                if kind == "e3":
                    for j in range(gck):
                        chunk_group[base + j] = pos
            # fw piece i -> schedule position after which to issue its dma
            fw_issue_at = {}
            for i in range(n_fw_pieces):
                pos = max(0, chunk_group[i * FW_PIECE] - FW_EARLY)
                fw_issue_at.setdefault(pos, []).append(i)

            # --- issue order: first bytes needed first ---
            # ramp group 0 adj, then fw piece 0, then the rest
            gidx = 0
            step = [0]  # global accumulation step

            def mm_e3(at, j_local, c_global, mb):
                piece, off = divmod(c_global, FW_PIECE)
                nc.tensor.matmul(
                    po[mb],
                    lhsT=fw_tiles[piece][:, off * F : (off + 1) * F],
                    rhs=at[:, j_local * ROWS + mb * 512 : j_local * ROWS + (mb + 1) * 512],
                    start=(step[0] == 0),
                    stop=(step[0] == total_ck - 1),
                )

            def mm_dr(at, j_local, c_global, mb):
                w = fwab_t[:, c_global * 2 * F : (c_global + 1) * 2 * F].rearrange(
                    "p (two f) -> p two f", two=2
                )
                rhs = at[:, j_local * 2 * ROWS : (j_local + 1) * 2 * ROWS].rearrange(
                    "p (two m) -> p two m", two=2
                )[:, :, mb * 512 : (mb + 1) * 512]
                nc.tensor.matmul(
                    po[mb],
                    lhsT=w,
                    rhs=rhs,
                    start=(step[0] == 0),
                    stop=(step[0] == total_ck - 1),
                    perf_mode=DRM,
                )

            # --- preamble ---
            # p-state warmup reads a tiny tile landing first on the fw ring;
            # dummy matmuls run at low/mid clock during boot so real matmuls
            # start at full speed. po[0] is reset by its later start=True.
            warm_t = const_pool.tile([P, 512], e3)
            nc.scalar.dma_start(out=warm_t, in_=warm[:])
            for _ in range(WARMUP):
                nc.tensor.matmul(
                    po[0], lhsT=warm_t[:, :128], rhs=warm_t[:], start=True, stop=True
                )
            fw_dma(0)
            fw_next[0] = 1
            nc.scalar.dma_start(out=b_sb, in_=bias[:])

            def adj3_dma(base, gck):
                at = pools[gck].tile([P, gck * ROWS], e3, name=f"a{gck}", tag=f"a{gck}")
                nc.sync.dma_start(
                    out=at, in_=adj3[:, base * ROWS : (base + gck) * ROWS]
                )
                return at

            for gi, (kind, base, gck) in enumerate(sched):
                last = gi == len(sched) - 1
                if kind == "e3":
                    at = adj3_dma(base, gck)
                    if gi == 8:
                        # fwab is small (0.44 MiB) and needed from the first
                        # DR group; issue a few groups early on the fw ring
                        nc.scalar.dma_start(out=fwab_t, in_=fwab[:])
                    for i in fw_issue_at.get(gi, []):
                        if i >= fw_next[0]:
                            fw_dma(i)
                            fw_next[0] = i + 1
                    if not last:
                        for j in range(gck):
                            for mb in range(4):
                                mm_e3(at, j, base + j, mb)
                            step[0] += 1
                    else:
                        sv = step[0]
                        for mb in range(4):
                            for j in range(gck):
                                step[0] = sv + j
                                mm_e3(at, j, base + j, mb)
                            sl = slice(mb * 512, (mb + 1) * 512)
                            # out dma rides the (drained) sync ring so the
                            # scalar sequencer runs the four ACTs back-to-back
                            # instead of serializing act->dispatch->act->...
                            nc.scalar.activation(
                                o_sb[:, sl],
                                po[mb],
                                mybir.ActivationFunctionType.Identity,
                                bias=b_sb,
                                scale=1.0 / (S * S),
                            )
                            nc.sync.dma_start(out=outT[:, sl], in_=o_sb[:, sl])
                        step[0] = sv + gck
                else:
                    at = adjdr_pool.tile(
                        [P, gck * 2 * ROWS], e4, name=f"adr{gck}", tag=f"adr{gck}"
                    )
                    nc.sync.dma_start(
                        out=at, in_=adj4[:, base * 2 * ROWS : (base + gck) * 2 * ROWS]
                    )
                    for i in fw_issue_at.get(gi, []):
                        if i >= fw_next[0]:
                            fw_dma(i)
                            fw_next[0] = i + 1
                    for j in range(gck):
                        for mb in range(4):
                            mm_dr(at, j, base + j, mb)
                        step[0] += 1
            assert step[0] == total_ck

    if split_waits:
        _split_excess_waits(nc)
    return nc


def _get_nc():
    if "nc" not in _cache:
        _cache["nc"] = _build()
    return _cache["nc"]


def make_in_maps(adj, features, W, b):
    adj = np.asarray(adj, dtype=np.float32)
    features = np.asarray(features, dtype=np.float32)
    W = np.asarray(W, dtype=np.float32)
    b = np.asarray(b, dtype=np.float32)

    fw = features @ W.T  # [N, F] f32
    sfw = fw * np.float32(S)
    fwab_rows = sfw[:KDR].astype(ml_dtypes.float8_e4m3)  # [KDR, F]
    fw16_rows = sfw[KDR:].astype(np.float16)  # [N-KDR, F]

    # fwab[p, c*2F + i*F + fo] = fwab_rows[c*256 + i*128 + p, fo]
    fwab = np.ascontiguousarray(
        fwab_rows.reshape(NDR, 2, P, F).transpose(2, 0, 1, 3)
    ).reshape(P, NDR * 2 * F)
    # fw16[p, c*F + fo] = fw16_rows[c*128 + p, fo]
    fw16 = np.ascontiguousarray(
        fw16_rows.reshape(E3C, P, F).transpose(1, 0, 2)
    ).reshape(P, E3C * F)

    # bias' = b + 0.5 * colsum(decoded effective fw), exact in f64
    fw_eff = np.concatenate(
        [
            fwab_rows.astype(np.float64) / S,
            fw16_rows.astype(np.float64) / S,
        ]
    )
    bias = (b.astype(np.float64) + 0.5 * fw_eff.sum(axis=0)).astype(np.float32)
    bias = np.ascontiguousarray(bias.reshape(P, 1))

    in_maps = []
    for c in range(CORES):
        sh = adj[c * ROWS : (c + 1) * ROWS, :]  # [ROWS, N]
        cen = (sh - np.float32(0.5)) * np.float32(S)
        q4 = cen[:, :KDR].astype(ml_dtypes.float8_e4m3)  # [ROWS, KDR]
        q3 = cen[:, KDR:].astype(ml_dtypes.float8_e3m4)  # [ROWS, N-KDR]
        # adj4[p, c*2*ROWS + i*ROWS + m] = q4.T[c*256 + i*128 + p, m]
        a4 = np.ascontiguousarray(
            q4.T.reshape(NDR, 2, P, ROWS).transpose(2, 0, 1, 3)
        ).reshape(P, NDR * 2 * ROWS)
        # adj3[p, c*ROWS + m] = q3.T[c*128 + p, m]
        a3 = np.ascontiguousarray(
            q3.T.reshape(E3C, P, ROWS).transpose(1, 0, 2)
        ).reshape(P, E3C * ROWS)
        in_maps.append(
            {
                "adj3": a3,
                "adj4": a4,
                "fw16": fw16,
                "fwab": fwab,
                "bias": bias,
                "warm": np.zeros((P, 512), ml_dtypes.float8_e3m4),
            }
        )
    return in_maps


def assemble_output(results):
    out = np.empty((N, F), dtype=np.float32)
    for c in range(CORES):
        out[c * ROWS : (c + 1) * ROWS, :] = results[c]["outT"].astype(np.float32).T
    return out


def kernel(adj, features, W, b):
    from concourse.bass_utils import run_bass_kernel_spmd

    nc = _get_nc()
    in_maps = make_in_maps(adj, features, W, b)
    res = run_bass_kernel_spmd(nc, in_maps, list(range(CORES)))
    return assemble_output(res.results)
